# revision 29
# baseline (speedup 1.0000x reference)
"""DeepseekV3 MLA attention on 8 Trainium2 NeuronCores (Bass/Tile).

Sharding: token-parallel. Core c handles batch b = c//4 and 512 query tokens
(4 chunks of 128, zig-zag {a, 7-a, 8+a, 15-a} with a = c%4, ordered by
descending causal-prefix length so every core runs the identical program).
Each core computes the full KV path for its batch (replicated within the
4-core batch group), attention for all 16 heads over its queries, and the
full o-projection for its tokens.  No collectives; host unsharding is pure
concatenation/transposition.

Layouts are transposed (feature-on-partition) end to end; weights arrive
host-pre-transposed; RoPE runs in transposed layout using partition-shifted
single-input copies; softmax is unstabilized exp (scores are ~N(0,1) after
the 1/sqrt(192) scaling, bounded far below f32 overflow); causal masking is
data-driven (per-slot mask tiles multiply probabilities after exp).
"""

import numpy as np

import concourse.bass as bass
import concourse.mybir as mybir
import concourse.tile as tile
from concourse import bacc
from concourse.masks import make_identity
from concourse.bass_utils import run_bass_kernel_spmd

# Problem constants (hardcoded per contract).
B, S, D = 2, 2048, 2048
H = 16
LQ = 1536           # q low-rank dim
LKV = 512           # kv latent dim
ROPE = 64
NOPE = 128
VDIM = 128
QKD = NOPE + ROPE   # 192
EPS = 1e-6
SCALING = QKD ** -0.5
NT = 512            # query tokens per core
NCORES = 8

F32 = mybir.dt.float32
F32R = mybir.dt.float32r
BF16 = mybir.dt.bfloat16
AF = mybir.ActivationFunctionType
ALU = mybir.AluOpType
AX = mybir.AxisListType


def _blocks_for(a: int) -> list[int]:
    """Query chunk (of 128 tokens) handled by col-block t, t=0..3.

    Block t has causal key prefix covering key groups kg=0..3-t (512 keys
    each); chunk ids are chosen so prefixes are compatible: J_t in
    [12-4t, 15-4t].
    """
    return [15 - a, 8 + a, 7 - a, a]


def build(nheads: int = H, mm_dt=BF16, p_dt=BF16):
    """Build the SPMD Bass program."""
    HP = nheads
    NPAIR = (HP + 1) // 2
    HG = (HP + 3) // 4
    GSZ = min(4, HP)
    NLQ = LQ // 128      # 12
    NLKV = LKV // 128    # 4

    nc = bacc.Bacc("TRN2", target_bir_lowering=False, debug=False,
                   num_devices=NCORES)

    def din(name, shape, dt=mm_dt):
        return nc.dram_tensor(name, list(shape), dt, kind="ExternalInput")

    hidT = din("hidT", [D, S])
    hidTq = din("hidTq", [D, NT])
    qaWt = din("qaWt", [D, LQ])
    qbWp = din("qbWp", [LQ, HP * NOPE])
    qbWr = din("qbWr", [LQ, HP * ROPE])
    kvaWt = din("kvaWt", [D, LKV + ROPE])
    kvbWk = din("kvbWk", [LKV, HP * NOPE])
    kvbWv = din("kvbWv", [LKV, HP * VDIM])
    oWt = din("oWt", [HP * VDIM, D])
    cosq2 = din("cosq2", [2 * ROPE, NT], F32)
    sinq2 = din("sinq2", [2 * ROPE, NT], F32)   # first half negated
    cosk = din("cosk", [ROPE, S], F32)
    sink = din("sink", [ROPE, S], F32)          # first half negated
    masks = din("masks", [16, 128, 128])
    outT = nc.dram_tensor("outT", [D, NT], F32, kind="ExternalOutput")

    with tile.TileContext(nc) as tc:
        with (
            tc.tile_pool(name="psum", bufs=1, space="PSUM") as pp,
            tc.tile_pool(name="peC", bufs=1) as peC,
            tc.tile_pool(name="small", bufs=1) as sm,
        ):
            def ps_score():
                return pp.tile([128, 512], F32, tag="score", bufs=2,
                               name="ps_score")

            def ps_pt(i, dt=F32):
                return pp.tile([128, 512], dt, tag=f"pt{i}", bufs=1,
                               name=f"ps_pt{i}")

            def ps_out():
                return pp.tile([128, 512], F32, tag="out", bufs=2,
                               name="ps_out")

            def ps_misc():
                return pp.tile([128, 512], F32, tag="misc", bufs=1,
                               name="ps_misc")

            def ps_tp(dt):
                return pp.tile([128, 512], dt, tag="score", bufs=2,
                               name="ps_tp")

            # identities, eps
            ident_f = sm.tile([128, 128], F32)
            make_identity(nc, ident_f[:])
            ident_r = sm.tile([128, 128], mm_dt)
            nc.vector.tensor_copy(ident_r[:], ident_f[:])
            epsb = sm.tile([128, 1], F32)
            nc.vector.memset(epsb[:], EPS)

            # Long-lived (through phase C) tensors.
            qp = peC.tile([128, HP, NT], mm_dt)      # q_pass^T per head
            qr = peC.tile([128, NPAIR, NT], mm_dt)   # roped q_rot^T head pairs
            krT = peC.tile([128, S], mm_dt)          # roped k_rot^T (dup halves)
            cT = peC.tile([128, NLKV, S], mm_dt)     # c-tilde^T
            at = peC.tile([128, HP, NT], mm_dt)      # attn out, head-major

            # ============ Phase A-q: q_a + rmsnorm + transpose ==========
            with tc.tile_pool(name="phaq", bufs=1) as ph:
                qT = ph.tile([128, NLQ, NT], mm_dt)
                # next-phase tiles, DMA-prefetched during A-q compute
                kvw = ph.tile([128, 16, LKV + ROPE], mm_dt)
                ck = ph.tile([ROPE, S], F32, tag="ck")
                sk = ph.tile([ROPE, S], F32, tag="sk")
                hid0 = ph.tile([128, 8, 512], mm_dt, tag="hid0")
                w0 = ph.tile([128, NLQ, 2 * NOPE], mm_dt, tag="qbw0")
                cq = ph.tile([2 * ROPE, NT], F32, tag="cq")
                sq2 = ph.tile([2 * ROPE, NT], F32, tag="sq2")
                with tc.tile_pool(name="phaq2", bufs=1) as ph2:
                    hq = ph2.tile([128, 16, NT], mm_dt)
                    nc.sync.dma_start(
                        hq[:], hidTq.rearrange("(dk p) t -> p dk t", p=128))
                    qnat = ph2.tile([128, 4, LQ], mm_dt)
                    ssq = ph2.tile([128, 4], F32)  # per-tq sum of squares
                    for g in range(3):
                        accs = [ps_pt(0), ps_pt(1), ps_pt(2), ps_out()]
                        for dk2 in range(8):
                            qaw = ph2.tile([128, 2, 512], mm_dt, tag="qaw",
                                           bufs=3)
                            nc.sync.dma_start(
                                qaw[:],
                                qaWt[dk2 * 256:(dk2 + 1) * 256,
                                     g * 512:(g + 1) * 512].rearrange(
                                    "(two p) c -> p two c", p=128))
                            for ti in range(2):
                                dk = dk2 * 2 + ti
                                for tq in range(4):
                                    nc.tensor.matmul(
                                        accs[tq][:],
                                        hq[:, dk, tq * 128:(tq + 1) * 128],
                                        qaw[:, ti, :],
                                        start=(dk == 0), stop=(dk == 15))
                        for tq in range(4):
                            sq = ph2.tile([128, 512], F32, tag="sq", bufs=2)
                            nc.scalar.activation(out=sq[:], in_=accs[tq][:],
                                                 func=AF.Square)
                            ps = sm.tile([128, 1], F32, tag="ss", bufs=4)
                            nc.vector.reduce_sum(out=ps[:], in_=sq[:],
                                                 axis=AX.X)
                            if g == 0:
                                nc.vector.tensor_copy(ssq[:, tq:tq + 1],
                                                      ps[:])
                            else:
                                nc.vector.tensor_add(ssq[:, tq:tq + 1],
                                                     ssq[:, tq:tq + 1], ps[:])
                            nc.vector.tensor_copy(
                                qnat[:, tq, g * 512:(g + 1) * 512], accs[tq][:])
                        if g == 0:
                            # prefetch A-kv inputs while g=1/2 compute
                            nc.sync.dma_start(
                                kvw[:],
                                kvaWt.rearrange("(dk p) c -> p dk c", p=128))
                            nc.sync.dma_start(
                                hid0[:],
                                hidT[0:1024, 0:512].rearrange(
                                    "(dk p) t -> p dk t", p=128))
                    # prefetch rope tables + B-q first weight tile
                    nc.sync.dma_start(ck[:], cosk[:])
                    nc.sync.dma_start(sk[:], sink[:])
                    nc.sync.dma_start(
                        w0[:],
                        qbWp[:, 0:2 * NOPE].rearrange("(lk p) c -> p lk c",
                                                      p=128))
                    nc.sync.dma_start(cq[:], cosq2[:])
                    nc.sync.dma_start(sq2[:], sinq2[:])
                    # rsqrt + scale + transpose
                    nc.scalar.activation(out=ssq[:], in_=ssq[:], func=AF.Sqrt,
                                         bias=epsb[:], scale=1.0 / LQ)
                    nc.vector.reciprocal(out=ssq[:], in_=ssq[:])
                    for tq in range(4):
                        nc.vector.tensor_scalar(
                            out=qnat[:, tq, :], in0=qnat[:, tq, :],
                            scalar1=ssq[:, tq:tq + 1], scalar2=None,
                            op0=ALU.mult)
                    for lk in range(NLQ):
                        tp = ps_tp(mm_dt)
                        for tq in range(4):
                            nc.tensor.transpose(
                                tp[:, tq * 128:(tq + 1) * 128],
                                qnat[:, tq, lk * 128:(lk + 1) * 128],
                                ident_r[:])
                        nc.scalar.copy(qT[:, lk, :], tp[:])

                # ==== Phase A-kv: kv_a + rmsnorm + k-rope + transpose ===
                with tc.tile_pool(name="phakv", bufs=1) as ph4:
                    for q4 in range(4):
                        accs = [ps_pt(0), ps_pt(1), ps_pt(2), ps_out()]
                        rot = ps_misc()
                        for half in range(2):
                            if q4 == 0 and half == 0:
                                hid = hid0
                            else:
                                hid = ph4.tile([128, 8, 512], mm_dt,
                                               tag="hid", bufs=2)
                                nc.sync.dma_start(
                                    hid[:],
                                    hidT[half * 1024:(half + 1) * 1024,
                                         q4 * 512:(q4 + 1) * 512].rearrange(
                                        "(dk p) t -> p dk t", p=128))
                            for dk8 in range(8):
                                dk = half * 8 + dk8
                                for tk in range(4):
                                    nc.tensor.matmul(
                                        accs[tk][:],
                                        hid[:, dk8, tk * 128:(tk + 1) * 128],
                                        kvw[:, dk, 0:LKV],
                                        start=(dk == 0), stop=(dk == 15))
                                nc.tensor.matmul(rot[0:ROPE, :],
                                                 kvw[:, dk, LKV:LKV + ROPE],
                                                 hid[:, dk8, :],
                                                 start=(dk == 0),
                                                 stop=(dk == 15))
                        for tk in range(4):
                            tt = q4 * 4 + tk
                            sq = ph4.tile([128, LKV], F32, tag="sqkv", bufs=2)
                            nc.scalar.activation(out=sq[:], in_=accs[tk][:],
                                                 func=AF.Square)
                            ss = sm.tile([128, 1], F32, tag="ss", bufs=4)
                            nc.vector.reduce_sum(out=ss[:], in_=sq[:],
                                                 axis=AX.X)
                            nc.scalar.activation(out=ss[:], in_=ss[:],
                                                 func=AF.Sqrt, bias=epsb[:],
                                                 scale=1.0 / LKV)
                            nc.vector.reciprocal(out=ss[:], in_=ss[:])
                            cnt = ph4.tile([128, LKV], mm_dt, tag="cnt",
                                           bufs=3)
                            nc.vector.tensor_scalar(
                                out=cnt[:], in0=accs[tk][:],
                                scalar1=ss[:], scalar2=None, op0=ALU.mult)
                            tp = ps_tp(mm_dt)
                            for lk in range(NLKV):
                                nc.tensor.transpose(
                                    tp[:, lk * 128:(lk + 1) * 128],
                                    cnt[:, lk * 128:(lk + 1) * 128],
                                    ident_r[:])
                            nc.scalar.copy(
                                cT[:, :, tt * 128:(tt + 1) * 128],
                                tp[:].rearrange("p (lk c) -> p lk c", c=128))
                        kR = ph4.tile([ROPE, 512], F32, tag="kR", bufs=2)
                        kS = ph4.tile([ROPE, 512], F32, tag="kS", bufs=2)
                        nc.scalar.copy(kR[:], rot[0:ROPE, :])
                        nc.scalar.copy(kS[0:32, :], rot[32:64, :])
                        nc.scalar.copy(kS[32:64, :], rot[0:32, :])
                        cs = ck[:, q4 * 512:(q4 + 1) * 512]
                        sn = sk[:, q4 * 512:(q4 + 1) * 512]
                        nc.vector.tensor_mul(kR[:], kR[:], cs)
                        nc.vector.tensor_mul(kS[:], kS[:], sn)
                        nc.vector.tensor_add(
                            krT[0:ROPE, q4 * 512:(q4 + 1) * 512],
                            kR[:], kS[:])
                        nc.vector.tensor_add(
                            krT[ROPE:2 * ROPE, q4 * 512:(q4 + 1) * 512],
                            kR[:], kS[:])

                # ============ Phase B-q: q_b + q-rope (uses qT) =========
                with tc.tile_pool(name="phbq", bufs=1) as ph3:
                    for h2 in range((HP + 1) // 2):
                        nh = min(2, HP - h2 * 2)
                        if h2 == 0:
                            w = w0
                        else:
                            w = ph3.tile([128, NLQ, 2 * NOPE], mm_dt,
                                         tag="qbw", bufs=3)
                            nc.sync.dma_start(
                                w[:, :, 0:nh * NOPE],
                                qbWp[:, h2 * 2 * NOPE:
                                     (h2 * 2 + nh) * NOPE].rearrange(
                                    "(lk p) c -> p lk c", p=128))
                        for hh in range(nh):
                            h = h2 * 2 + hh
                            acc = ps_score()
                            for lk in range(NLQ):
                                nc.tensor.matmul(
                                    acc[:],
                                    w[:, lk, hh * NOPE:(hh + 1) * NOPE],
                                    qT[:, lk, :],
                                    start=(lk == 0), stop=(lk == NLQ - 1))
                            if h % 2 == 0:
                                nc.scalar.copy(qp[:, h, :], acc[:])
                            else:
                                nc.vector.tensor_copy(qp[:, h, :], acc[:])
                    for p in range(NPAIR):
                        npr = min(2 * ROPE, HP * ROPE - p * 2 * ROPE)
                        w = ph3.tile([128, NLQ, 2 * ROPE], mm_dt, tag="qrw",
                                     bufs=3)
                        nc.sync.dma_start(
                            w[:, :, 0:npr],
                            qbWr[:, p * 2 * ROPE:p * 2 * ROPE + npr].rearrange(
                                "(lk p) c -> p lk c", p=128))
                        acc = ps_score()
                        for lk in range(NLQ):
                            nc.tensor.matmul(acc[0:npr, :], w[:, lk, 0:npr],
                                             qT[:, lk, :],
                                             start=(lk == 0),
                                             stop=(lk == NLQ - 1))
                        qR = ph3.tile([128, NT], F32, tag="qR", bufs=2)
                        qS = ph3.tile([128, NT], F32, tag="qS", bufs=2)
                        for hh in range(npr // ROPE):
                            r0 = hh * ROPE
                            nc.scalar.copy(qS[r0:r0 + 32, :],
                                           acc[r0 + 32:r0 + 64, :])
                            nc.scalar.copy(qS[r0 + 32:r0 + 64, :],
                                           acc[r0:r0 + 32, :])
                        nc.vector.tensor_mul(qR[0:npr, :], acc[0:npr, :],
                                             cq[0:npr, :])
                        nc.vector.tensor_mul(qS[0:npr, :], qS[0:npr, :],
                                             sq2[0:npr, :])
                        nc.vector.tensor_add(qr[0:npr, p, :], qR[0:npr, :],
                                             qS[0:npr, :])

            # ============ Phase C: attention ============================
            # Transposed scores: S^T[k, q] = K @ Q^T per 128-key tile; AV
            # consumes P~^T directly (lhsT = [V-half | ones]), yielding the
            # softmax denominator as PSUM row 64 for free.
            with tc.tile_pool(name="phc", bufs=1) as ph:
                msk = ph.tile([128, 16, 128], mm_dt)
                nc.sync.dma_start(msk[:], masks.rearrange("t p c -> p t c"))
                ones1 = sm.tile([128, 1], F32)
                nc.vector.memset(ones1[:], 1.0)
                ow0 = ph.tile([128, HP, 512], mm_dt, tag="ow0")
                for hg in range(HG):
                    if hg == 1:
                        nc.sync.dma_start(
                            ow0[:],
                            oWt[:, 0:512].rearrange("(hk p) c -> p hk c",
                                                    p=128))
                    h0 = hg * GSZ
                    gs = min(GSZ, HP - h0)
                    vw = ph.tile([128, NLKV, GSZ * VDIM], mm_dt,
                                 tag="vw", bufs=2)
                    nc.sync.dma_start(
                        vw[:, :, 0:gs * VDIM],
                        kvbWv[:, h0 * VDIM:(h0 + gs) * VDIM].rearrange(
                            "(lk p) c -> p lk c", p=128))
                    kwg = ph.tile([128, NLKV, GSZ * NOPE], mm_dt,
                                  tag="kwg", bufs=2)
                    nc.sync.dma_start(
                        kwg[:, :, 0:gs * NOPE],
                        kvbWk[:, h0 * NOPE:(h0 + gs) * NOPE].rearrange(
                            "(lk p) c -> p lk c", p=128))
                    v4 = ph.tile([128, 16, GSZ, 2, 65], mm_dt, tag="v4",
                                 bufs=1)
                    nc.vector.tensor_copy(
                        v4[:, :, 0:gs, :, 64:65],
                        ones1[:].to_broadcast([128, 16, gs, 2, 1]))
                    for tt in range(16):
                        acc = ps_misc() if tt % 2 == 0 else ps_pt(2)
                        for lk in range(NLKV):
                            nc.tensor.matmul(
                                acc[:, 0:gs * VDIM],
                                cT[:, lk, tt * 128:(tt + 1) * 128],
                                vw[:, lk, 0:gs * VDIM],
                                start=(lk == 0), stop=(lk == NLKV - 1))
                        nc.scalar.copy(
                            v4[:, tt, 0:gs, :, 0:64],
                            acc[:, 0:gs * VDIM].rearrange(
                                "p (g two s) -> p g two s", two=2, s=64))
                    for hh in range(gs):
                        h = h0 + hh
                        kh = ph.tile([128, S], mm_dt, tag="kh", bufs=2)
                        for kg4 in range(4):
                            acc = ps_misc() if kg4 % 2 == 0 else ps_pt(2)
                            for lk in range(NLKV):
                                nc.tensor.matmul(
                                    acc[:],
                                    kwg[:, lk, hh * NOPE:(hh + 1) * NOPE],
                                    cT[:, lk, kg4 * 512:(kg4 + 1) * 512],
                                    start=(lk == 0), stop=(lk == NLKV - 1))
                            nc.vector.tensor_copy(
                                kh[:, kg4 * 512:(kg4 + 1) * 512], acc[:])
                        r0 = (h % 2) * ROPE
                        if h % 2 == 0:
                            oacc1 = ps_out()
                            oacc2 = ps_out()
                        else:
                            oacc1 = ps_pt(0)
                            oacc2 = ps_pt(1)
                        for kt in range(16):
                            nb = 4 - kt // 4
                            N = nb * 128
                            sc = ps_score()
                            nc.tensor.matmul(
                                sc[:, 0:N],
                                kh[:, kt * 128:(kt + 1) * 128],
                                qp[:, h, 0:N],
                                start=True, stop=False)
                            nc.tensor.matmul(
                                sc[:, 0:N],
                                krT[r0:r0 + ROPE, kt * 128:(kt + 1) * 128],
                                qr[r0:r0 + ROPE, h // 2, 0:N],
                                start=False, stop=True)
                            pt_ = ph.tile([128, 512], p_dt, tag="P", bufs=3)
                            nc.scalar.activation(out=pt_[:, 0:N],
                                                 in_=sc[:, 0:N], func=AF.Exp,
                                                 scale=SCALING)
                            nc.vector.tensor_mul(pt_[:, N - 128:N],
                                                 pt_[:, N - 128:N],
                                                 msk[:, kt, :])
                            nc.tensor.matmul(
                                oacc1[0:65, 0:N],
                                v4[:, kt, hh, 0, :],
                                pt_[:, 0:N],
                                start=(kt == 0), stop=(kt == 15),
                                skip_group_check=True)
                            nc.tensor.matmul(
                                oacc2[0:65, 0:N],
                                v4[:, kt, hh, 1, :],
                                pt_[:, 0:N],
                                start=(kt == 0), stop=(kt == 15),
                                skip_group_check=True)
                        srow = sm.tile([1, 512], F32, tag="srow", bufs=2)
                        nc.scalar.copy(srow[:], oacc1[64:65, :])
                        nc.vector.reciprocal(out=srow[:], in_=srow[:])
                        ib = ph.tile([128, 512], F32, tag="ib", bufs=2)
                        nc.gpsimd.partition_broadcast(ib[:], srow[0:1, :])
                        nc.vector.tensor_mul(at[0:64, h, :], oacc1[0:64, :],
                                             ib[0:64, :])
                        nc.vector.tensor_mul(at[64:128, h, :], oacc2[0:64, :],
                                             ib[0:64, :])

                # ============ Phase D: o-projection =====================
                for dq in range(4):
                    if dq == 0:
                        ow = ow0
                    else:
                        ow = ph.tile([128, HP, 512], mm_dt, tag="ow", bufs=2)
                        nc.sync.dma_start(
                            ow[:],
                            oWt[:, dq * 512:(dq + 1) * 512].rearrange(
                                "(hk p) c -> p hk c", p=128))
                    accs = [ps_pt(0), ps_pt(1), ps_pt(2), ps_out()]
                    for dt in range(4):
                        for hk in range(HP):
                            nc.tensor.matmul(
                                accs[dt][:],
                                ow[:, hk, dt * 128:(dt + 1) * 128],
                                at[:, hk, :],
                                start=(hk == 0), stop=(hk == HP - 1))
                    for dt in range(4):
                        ot = ph.tile([128, 512], F32, tag="ot", bufs=3)
                        if dt % 2 == 0:
                            nc.scalar.copy(ot[:], accs[dt][:])
                        else:
                            nc.vector.tensor_copy(ot[:], accs[dt][:])
                        d0 = dq * 512 + dt * 128
                        nc.sync.dma_start(outT[d0:d0 + 128, :], ot[:])

    nc.compile()
    return nc


def prep_inputs(inputs: dict, nheads: int = H) -> list[dict]:
    """Shard + pre-transpose the full inputs into 8 per-core input maps."""
    import ml_dtypes
    bf16 = ml_dtypes.bfloat16
    f32 = np.float32
    hs = np.asarray(inputs["hidden_states"], f32)
    cos = np.asarray(inputs["cos"], f32)
    sin = np.asarray(inputs["sin"], f32)
    qaW = np.asarray(inputs["q_a_W"], f32)
    qanw = np.asarray(inputs["q_a_norm_w"], f32)
    qbW = np.asarray(inputs["q_b_W"], f32)
    kvaW = np.asarray(inputs["kv_a_W"], f32)
    kvanw = np.asarray(inputs["kv_a_norm_w"], f32)
    kvbW = np.asarray(inputs["kv_b_W"], f32)
    oW = np.asarray(inputs["o_W"], f32)

    HP = nheads
    qaWt = np.ascontiguousarray(qaW.T)                      # [D, LQ]
    # fold q_a_norm_w into q_b rows (columns of q_b_W)
    qbWs = qbW[: HP * QKD] * qanw[None, :]                  # [HP*QKD, LQ]
    qb3 = qbWs.reshape(HP, QKD, LQ)
    qbWp = np.ascontiguousarray(
        qb3[:, :NOPE, :].reshape(HP * NOPE, LQ).T)          # [LQ, HP*128]
    qbWr = np.ascontiguousarray(
        qb3[:, NOPE:, :].reshape(HP * ROPE, LQ).T)          # [LQ, HP*64]
    kvaWt = np.ascontiguousarray(kvaW.T)                    # [D, 576]
    kvb3 = (kvbW[: HP * (NOPE + VDIM)] * kvanw[None, :]).reshape(
        HP, NOPE + VDIM, LKV)
    kvbWk = np.ascontiguousarray(
        kvb3[:, :NOPE, :].reshape(HP * NOPE, LKV).T)        # [LKV, HP*128]
    kvbWv = np.ascontiguousarray(
        kvb3[:, NOPE:, :].reshape(HP * VDIM, LKV).T)        # [LKV, HP*128]
    oWt = np.ascontiguousarray(oW[:, : HP * VDIM].T)        # [HP*128, D]

    qaWt = qaWt.astype(bf16)
    qbWp = qbWp.astype(bf16)
    qbWr = qbWr.astype(bf16)
    kvaWt = kvaWt.astype(bf16)
    kvbWk = kvbWk.astype(bf16)
    kvbWv = kvbWv.astype(bf16)
    oWt = oWt.astype(bf16)
    hTb = [np.ascontiguousarray(hs[b].T).astype(bf16) for b in range(B)]

    in_maps = []
    for c in range(NCORES):
        b, a = divmod(c, 4)
        blocks = _blocks_for(a)
        qidx = np.concatenate(
            [np.arange(j * 128, (j + 1) * 128) for j in blocks])
        hT = hTb[b]                                         # [D, S] bf16
        cosT = np.ascontiguousarray(cos[b].T)               # [64, S]
        sinT = np.ascontiguousarray(sin[b].T)
        sinneg = sinT.copy()
        sinneg[:32] = -sinneg[:32]
        cosq = cosT[:, qidx]
        sinq = sinneg[:, qidx]
        hTq = np.ascontiguousarray(hT[:, qidx])             # [D, NT] bf16
        mk = np.zeros((16, 128, 128), f32)
        for kt in range(16):
            t = 3 - kt // 4
            j = blocks[t]
            qpos = np.arange(j * 128, (j + 1) * 128)
            kpos = kt * 128 + np.arange(128)
            mk[kt] = (kpos[:, None] <= qpos[None, :]).astype(f32)
        in_maps.append({
            "hidT": hT,
            "hidTq": hTq,
            "qaWt": qaWt, "qbWp": qbWp, "qbWr": qbWr,
            "kvaWt": kvaWt, "kvbWk": kvbWk, "kvbWv": kvbWv, "oWt": oWt,
            "cosq2": np.ascontiguousarray(np.concatenate([cosq, cosq], 0)),
            "sinq2": np.ascontiguousarray(np.concatenate([sinq, sinq], 0)),
            "cosk": cosT, "sink": sinneg,
            "masks": mk.astype(bf16),
        })
    return in_maps


def assemble(results: list[dict]) -> np.ndarray:
    out = np.empty((B, S, D), np.float32)
    for c in range(NCORES):
        b, a = divmod(c, 4)
        blocks = _blocks_for(a)
        oT = results[c]["outT"]                             # [D, 512]
        for t, j in enumerate(blocks):
            out[b, j * 128:(j + 1) * 128, :] = oT[:, t * 128:(t + 1) * 128].T
    return out


_CACHE = {}


def _get_nc(nheads=H):
    key = nheads
    if key not in _CACHE:
        _CACHE[key] = build(nheads)
    return _CACHE[key]


def kernel(**inputs) -> np.ndarray:
    nc = _get_nc()
    in_maps = prep_inputs(inputs)
    res = run_bass_kernel_spmd(nc, in_maps, list(range(NCORES)))
    return assemble(res.results)



# revision 32
# speedup vs baseline: 1.0090x; 1.0090x over previous
"""DeepseekV3 MLA attention on 8 Trainium2 NeuronCores (Bass/Tile).

Sharding: token-parallel. Core c handles batch b = c//4 and 512 query tokens
(4 chunks of 128, zig-zag {a, 7-a, 8+a, 15-a} with a = c%4, ordered by
descending causal-prefix length so every core runs the identical program).
Each core computes the full KV path for its batch (replicated within the
4-core batch group), attention for all 16 heads over its queries, and the
full o-projection for its tokens.  No collectives; host unsharding is pure
concatenation/transposition.

Layouts are transposed (feature-on-partition) end to end; weights arrive
host-pre-transposed; RoPE runs in transposed layout using partition-shifted
single-input copies; softmax is unstabilized exp (scores are ~N(0,1) after
the 1/sqrt(192) scaling, bounded far below f32 overflow); causal masking is
data-driven (per-slot mask tiles multiply probabilities after exp).
"""

import numpy as np

import concourse.bass as bass
import concourse.mybir as mybir
import concourse.tile as tile
from concourse import bacc
from concourse.masks import make_identity
from concourse.bass_utils import run_bass_kernel_spmd

# Problem constants (hardcoded per contract).
B, S, D = 2, 2048, 2048
H = 16
LQ = 1536           # q low-rank dim
LKV = 512           # kv latent dim
ROPE = 64
NOPE = 128
VDIM = 128
QKD = NOPE + ROPE   # 192
EPS = 1e-6
SCALING = QKD ** -0.5
NT = 512            # query tokens per core
NCORES = 8

F32 = mybir.dt.float32
F32R = mybir.dt.float32r
BF16 = mybir.dt.bfloat16
AF = mybir.ActivationFunctionType
ALU = mybir.AluOpType
AX = mybir.AxisListType


def _blocks_for(a: int) -> list[int]:
    """Query chunk (of 128 tokens) handled by col-block t, t=0..3.

    Block t has causal key prefix covering key groups kg=0..3-t (512 keys
    each); chunk ids are chosen so prefixes are compatible: J_t in
    [12-4t, 15-4t].
    """
    return [15 - a, 8 + a, 7 - a, a]


def build(nheads: int = H, mm_dt=BF16, p_dt=BF16):
    """Build the SPMD Bass program."""
    HP = nheads
    NPAIR = (HP + 1) // 2
    HG = (HP + 3) // 4
    GSZ = min(4, HP)
    NLQ = LQ // 128      # 12
    NLKV = LKV // 128    # 4

    nc = bacc.Bacc("TRN2", target_bir_lowering=False, debug=False,
                   num_devices=NCORES)

    def din(name, shape, dt=mm_dt):
        return nc.dram_tensor(name, list(shape), dt, kind="ExternalInput")

    hidT = din("hidT", [D, S])
    hidTq = din("hidTq", [D, NT])
    qaWt = din("qaWt", [D, LQ])
    qbWp = din("qbWp", [LQ, HP * NOPE])
    qbWr = din("qbWr", [LQ, HP * ROPE])
    kvaWt = din("kvaWt", [D, LKV + ROPE])
    kvbWk = din("kvbWk", [LKV, HP * NOPE])
    kvbWv = din("kvbWv", [LKV, HP * VDIM])
    oWt = din("oWt", [HP * VDIM, D])
    cosq2 = din("cosq2", [2 * ROPE, NT], F32)
    sinq2 = din("sinq2", [2 * ROPE, NT], F32)   # first half negated
    cosk = din("cosk", [ROPE, S], F32)
    sink = din("sink", [ROPE, S], F32)          # first half negated
    masks = din("masks", [16, 128, 128])
    outT = nc.dram_tensor("outT", [D, NT], F32, kind="ExternalOutput")

    with tile.TileContext(nc) as tc:
        with (
            tc.tile_pool(name="psum", bufs=1, space="PSUM") as pp,
            tc.tile_pool(name="peC", bufs=1) as peC,
            tc.tile_pool(name="small", bufs=1) as sm,
        ):
            def ps_score():
                return pp.tile([128, 512], F32, tag="score", bufs=2,
                               name="ps_score")

            def ps_pt(i, dt=F32):
                return pp.tile([128, 512], dt, tag=f"pt{i}", bufs=1,
                               name=f"ps_pt{i}")

            def ps_out():
                return pp.tile([128, 512], F32, tag="out", bufs=2,
                               name="ps_out")

            def ps_misc():
                return pp.tile([128, 512], F32, tag="misc", bufs=1,
                               name="ps_misc")

            def ps_tp(dt):
                return pp.tile([128, 512], dt, tag="score", bufs=2,
                               name="ps_tp")

            # identities, eps
            ident_f = sm.tile([128, 128], F32)
            make_identity(nc, ident_f[:])
            ident_r = sm.tile([128, 128], mm_dt)
            nc.vector.tensor_copy(ident_r[:], ident_f[:])
            epsb = sm.tile([128, 1], F32)
            nc.vector.memset(epsb[:], EPS)

            # Long-lived (through phase C) tensors.
            qp = peC.tile([128, HP, NT], mm_dt)      # q_pass^T per head
            vw0 = peC.tile([128, NLKV, GSZ * VDIM], mm_dt)   # C hg=0 V wts
            kwg0 = peC.tile([128, NLKV, GSZ * NOPE], mm_dt)  # C hg=0 K wts
            qr = peC.tile([128, NPAIR, NT], mm_dt)   # roped q_rot^T head pairs
            krT = peC.tile([128, S], mm_dt)          # roped k_rot^T (dup halves)
            cT = peC.tile([128, NLKV, S], mm_dt)     # c-tilde^T
            at = peC.tile([128, HP, NT], mm_dt)      # attn out, head-major

            # ============ Phase A-q: q_a + rmsnorm + transpose ==========
            with tc.tile_pool(name="phaq", bufs=1) as ph:
                qT = ph.tile([128, NLQ, NT], mm_dt)
                # next-phase tiles, DMA-prefetched during A-q compute
                kvw = ph.tile([128, 16, LKV + ROPE], mm_dt)
                ck = ph.tile([ROPE, S], F32, tag="ck")
                sk = ph.tile([ROPE, S], F32, tag="sk")
                hid0 = ph.tile([128, 8, 512], mm_dt, tag="hid0")
                w0 = ph.tile([128, NLQ, 2 * NOPE], mm_dt, tag="qbw0")
                cq = ph.tile([2 * ROPE, NT], F32, tag="cq")
                sq2 = ph.tile([2 * ROPE, NT], F32, tag="sq2")
                with tc.tile_pool(name="phaq2", bufs=1) as ph2:
                    hq = ph2.tile([128, 16, NT], mm_dt)
                    nc.sync.dma_start(
                        hq[:], hidTq.rearrange("(dk p) t -> p dk t", p=128))
                    qnat = ph2.tile([128, 4, LQ], mm_dt)
                    ssq = ph2.tile([128, 4], F32)  # per-tq sum of squares
                    for g in range(3):
                        accs = [ps_pt(0), ps_pt(1), ps_pt(2), ps_out()]
                        for dk2 in range(8):
                            qaw = ph2.tile([128, 2, 512], mm_dt, tag="qaw",
                                           bufs=3)
                            nc.sync.dma_start(
                                qaw[:],
                                qaWt[dk2 * 256:(dk2 + 1) * 256,
                                     g * 512:(g + 1) * 512].rearrange(
                                    "(two p) c -> p two c", p=128))
                            for ti in range(2):
                                dk = dk2 * 2 + ti
                                for tq in range(4):
                                    nc.tensor.matmul(
                                        accs[tq][:],
                                        hq[:, dk, tq * 128:(tq + 1) * 128],
                                        qaw[:, ti, :],
                                        start=(dk == 0), stop=(dk == 15))
                        for tq in range(4):
                            sq = ph2.tile([128, 512], F32, tag="sq", bufs=2)
                            nc.scalar.activation(out=sq[:], in_=accs[tq][:],
                                                 func=AF.Square)
                            ps = sm.tile([128, 1], F32, tag="ss", bufs=4)
                            nc.vector.reduce_sum(out=ps[:], in_=sq[:],
                                                 axis=AX.X)
                            if g == 0:
                                nc.vector.tensor_copy(ssq[:, tq:tq + 1],
                                                      ps[:])
                            else:
                                nc.vector.tensor_add(ssq[:, tq:tq + 1],
                                                     ssq[:, tq:tq + 1], ps[:])
                            nc.vector.tensor_copy(
                                qnat[:, tq, g * 512:(g + 1) * 512], accs[tq][:])
                        if g == 0:
                            # prefetch A-kv inputs while g=1/2 compute
                            nc.sync.dma_start(
                                kvw[:],
                                kvaWt.rearrange("(dk p) c -> p dk c", p=128))
                            nc.sync.dma_start(
                                hid0[:],
                                hidT[0:1024, 0:512].rearrange(
                                    "(dk p) t -> p dk t", p=128))
                    # prefetch rope tables + B-q first weight tile
                    nc.sync.dma_start(ck[:], cosk[:])
                    nc.sync.dma_start(sk[:], sink[:])
                    nc.sync.dma_start(
                        w0[:],
                        qbWp[:, 0:2 * NOPE].rearrange("(lk p) c -> p lk c",
                                                      p=128))
                    nc.sync.dma_start(cq[:], cosq2[:])
                    nc.sync.dma_start(sq2[:], sinq2[:])
                    # rsqrt + scale + transpose
                    nc.scalar.activation(out=ssq[:], in_=ssq[:], func=AF.Sqrt,
                                         bias=epsb[:], scale=1.0 / LQ)
                    nc.vector.reciprocal(out=ssq[:], in_=ssq[:])
                    for tq in range(4):
                        nc.vector.tensor_scalar(
                            out=qnat[:, tq, :], in0=qnat[:, tq, :],
                            scalar1=ssq[:, tq:tq + 1], scalar2=None,
                            op0=ALU.mult)
                    for lk in range(NLQ):
                        tp = ps_tp(mm_dt)
                        for tq in range(4):
                            nc.tensor.transpose(
                                tp[:, tq * 128:(tq + 1) * 128],
                                qnat[:, tq, lk * 128:(lk + 1) * 128],
                                ident_r[:])
                        nc.scalar.copy(qT[:, lk, :], tp[:])

                # ==== Phase A-kv: kv_a + rmsnorm + k-rope + transpose ===
                with tc.tile_pool(name="phakv", bufs=1) as ph4:
                    for q4 in range(4):
                        accs = [ps_pt(0), ps_pt(1), ps_pt(2), ps_out()]
                        rot = ps_misc()
                        for half in range(2):
                            if q4 == 0 and half == 0:
                                hid = hid0
                            else:
                                hid = ph4.tile([128, 8, 512], mm_dt,
                                               tag="hid", bufs=2)
                                nc.sync.dma_start(
                                    hid[:],
                                    hidT[half * 1024:(half + 1) * 1024,
                                         q4 * 512:(q4 + 1) * 512].rearrange(
                                        "(dk p) t -> p dk t", p=128))
                            for dk8 in range(8):
                                dk = half * 8 + dk8
                                for tk in range(4):
                                    nc.tensor.matmul(
                                        accs[tk][:],
                                        hid[:, dk8, tk * 128:(tk + 1) * 128],
                                        kvw[:, dk, 0:LKV],
                                        start=(dk == 0), stop=(dk == 15))
                                nc.tensor.matmul(rot[0:ROPE, :],
                                                 kvw[:, dk, LKV:LKV + ROPE],
                                                 hid[:, dk8, :],
                                                 start=(dk == 0),
                                                 stop=(dk == 15))
                        for tk in range(4):
                            tt = q4 * 4 + tk
                            sq = ph4.tile([128, LKV], F32, tag="sqkv", bufs=2)
                            nc.scalar.activation(out=sq[:], in_=accs[tk][:],
                                                 func=AF.Square)
                            ss = sm.tile([128, 1], F32, tag="ss", bufs=4)
                            nc.vector.reduce_sum(out=ss[:], in_=sq[:],
                                                 axis=AX.X)
                            nc.scalar.activation(out=ss[:], in_=ss[:],
                                                 func=AF.Sqrt, bias=epsb[:],
                                                 scale=1.0 / LKV)
                            nc.vector.reciprocal(out=ss[:], in_=ss[:])
                            cnt = ph4.tile([128, LKV], mm_dt, tag="cnt",
                                           bufs=3)
                            nc.vector.tensor_scalar(
                                out=cnt[:], in0=accs[tk][:],
                                scalar1=ss[:], scalar2=None, op0=ALU.mult)
                            tp = ps_tp(mm_dt)
                            for lk in range(NLKV):
                                nc.tensor.transpose(
                                    tp[:, lk * 128:(lk + 1) * 128],
                                    cnt[:, lk * 128:(lk + 1) * 128],
                                    ident_r[:])
                            nc.scalar.copy(
                                cT[:, :, tt * 128:(tt + 1) * 128],
                                tp[:].rearrange("p (lk c) -> p lk c", c=128))
                        kR = ph4.tile([ROPE, 512], F32, tag="kR", bufs=2)
                        kS = ph4.tile([ROPE, 512], F32, tag="kS", bufs=2)
                        nc.scalar.copy(kR[:], rot[0:ROPE, :])
                        nc.scalar.copy(kS[0:32, :], rot[32:64, :])
                        nc.scalar.copy(kS[32:64, :], rot[0:32, :])
                        cs = ck[:, q4 * 512:(q4 + 1) * 512]
                        sn = sk[:, q4 * 512:(q4 + 1) * 512]
                        nc.vector.tensor_mul(kR[:], kR[:], cs)
                        nc.vector.tensor_mul(kS[:], kS[:], sn)
                        nc.vector.tensor_add(
                            krT[0:ROPE, q4 * 512:(q4 + 1) * 512],
                            kR[:], kS[:])
                        nc.vector.tensor_add(
                            krT[ROPE:2 * ROPE, q4 * 512:(q4 + 1) * 512],
                            kR[:], kS[:])

                # ============ Phase B-q: q_b + q-rope (uses qT) =========
                with tc.tile_pool(name="phbq", bufs=1) as ph3:
                    for h2 in range((HP + 1) // 2):
                        nh = min(2, HP - h2 * 2)
                        if h2 == 0:
                            w = w0
                        else:
                            w = ph3.tile([128, NLQ, 2 * NOPE], mm_dt,
                                         tag="qbw", bufs=3)
                            nc.sync.dma_start(
                                w[:, :, 0:nh * NOPE],
                                qbWp[:, h2 * 2 * NOPE:
                                     (h2 * 2 + nh) * NOPE].rearrange(
                                    "(lk p) c -> p lk c", p=128))
                        for hh in range(nh):
                            h = h2 * 2 + hh
                            acc = ps_score()
                            for lk in range(NLQ):
                                nc.tensor.matmul(
                                    acc[:],
                                    w[:, lk, hh * NOPE:(hh + 1) * NOPE],
                                    qT[:, lk, :],
                                    start=(lk == 0), stop=(lk == NLQ - 1))
                            if h % 2 == 0:
                                nc.scalar.copy(qp[:, h, :], acc[:])
                            else:
                                nc.vector.tensor_copy(qp[:, h, :], acc[:])
                    for p in range(NPAIR):
                        npr = min(2 * ROPE, HP * ROPE - p * 2 * ROPE)
                        w = ph3.tile([128, NLQ, 2 * ROPE], mm_dt, tag="qrw",
                                     bufs=3)
                        nc.sync.dma_start(
                            w[:, :, 0:npr],
                            qbWr[:, p * 2 * ROPE:p * 2 * ROPE + npr].rearrange(
                                "(lk p) c -> p lk c", p=128))
                        acc = ps_score()
                        for lk in range(NLQ):
                            nc.tensor.matmul(acc[0:npr, :], w[:, lk, 0:npr],
                                             qT[:, lk, :],
                                             start=(lk == 0),
                                             stop=(lk == NLQ - 1))
                        qR = ph3.tile([128, NT], F32, tag="qR", bufs=2)
                        qS = ph3.tile([128, NT], F32, tag="qS", bufs=2)
                        for hh in range(npr // ROPE):
                            r0 = hh * ROPE
                            nc.scalar.copy(qS[r0:r0 + 32, :],
                                           acc[r0 + 32:r0 + 64, :])
                            nc.scalar.copy(qS[r0 + 32:r0 + 64, :],
                                           acc[r0:r0 + 32, :])
                        nc.vector.tensor_mul(qR[0:npr, :], acc[0:npr, :],
                                             cq[0:npr, :])
                        nc.vector.tensor_mul(qS[0:npr, :], qS[0:npr, :],
                                             sq2[0:npr, :])
                        nc.vector.tensor_add(qr[0:npr, p, :], qR[0:npr, :],
                                             qS[0:npr, :])
                    # prefetch phase-C first head-group weights
                    nc.sync.dma_start(
                        vw0[:],
                        kvbWv[:, 0:GSZ * VDIM].rearrange(
                            "(lk p) c -> p lk c", p=128))
                    nc.sync.dma_start(
                        kwg0[:],
                        kvbWk[:, 0:GSZ * NOPE].rearrange(
                            "(lk p) c -> p lk c", p=128))

            # ============ Phase C: attention ============================
            # Transposed scores: S^T[k, q] = K @ Q^T per 128-key tile; AV
            # consumes P~^T directly (lhsT = [V-half | ones]), yielding the
            # softmax denominator as PSUM row 64 for free.
            with tc.tile_pool(name="phc", bufs=1) as ph:
                msk = ph.tile([128, 16, 128], mm_dt)
                nc.sync.dma_start(msk[:], masks.rearrange("t p c -> p t c"))
                ones1 = sm.tile([128, 1], F32)
                nc.vector.memset(ones1[:], 1.0)
                ow0 = ph.tile([128, HP, 512], mm_dt, tag="ow0")
                for hg in range(HG):
                    if hg == 1:
                        nc.sync.dma_start(
                            ow0[:],
                            oWt[:, 0:512].rearrange("(hk p) c -> p hk c",
                                                    p=128))
                    h0 = hg * GSZ
                    gs = min(GSZ, HP - h0)
                    if hg == 0:
                        vw = vw0
                        kwg = kwg0
                    else:
                        vw = ph.tile([128, NLKV, GSZ * VDIM], mm_dt,
                                     tag="vw", bufs=2)
                        nc.sync.dma_start(
                            vw[:, :, 0:gs * VDIM],
                            kvbWv[:, h0 * VDIM:(h0 + gs) * VDIM].rearrange(
                                "(lk p) c -> p lk c", p=128))
                        kwg = ph.tile([128, NLKV, GSZ * NOPE], mm_dt,
                                      tag="kwg", bufs=2)
                        nc.sync.dma_start(
                            kwg[:, :, 0:gs * NOPE],
                            kvbWk[:, h0 * NOPE:(h0 + gs) * NOPE].rearrange(
                                "(lk p) c -> p lk c", p=128))
                    v4 = ph.tile([128, 16, GSZ, 2, 65], mm_dt, tag="v4",
                                 bufs=1)
                    nc.vector.tensor_copy(
                        v4[:, :, 0:gs, :, 64:65],
                        ones1[:].to_broadcast([128, 16, gs, 2, 1]))
                    for tt in range(16):
                        acc = ps_misc() if tt % 2 == 0 else ps_pt(2)
                        for lk in range(NLKV):
                            nc.tensor.matmul(
                                acc[:, 0:gs * VDIM],
                                cT[:, lk, tt * 128:(tt + 1) * 128],
                                vw[:, lk, 0:gs * VDIM],
                                start=(lk == 0), stop=(lk == NLKV - 1))
                        nc.scalar.copy(
                            v4[:, tt, 0:gs, :, 0:64],
                            acc[:, 0:gs * VDIM].rearrange(
                                "p (g two s) -> p g two s", two=2, s=64))
                    for hh in range(gs):
                        h = h0 + hh
                        kh = ph.tile([128, S], mm_dt, tag="kh", bufs=2)
                        for kg4 in range(4):
                            acc = ps_misc() if kg4 % 2 == 0 else ps_pt(2)
                            for lk in range(NLKV):
                                nc.tensor.matmul(
                                    acc[:],
                                    kwg[:, lk, hh * NOPE:(hh + 1) * NOPE],
                                    cT[:, lk, kg4 * 512:(kg4 + 1) * 512],
                                    start=(lk == 0), stop=(lk == NLKV - 1))
                            nc.vector.tensor_copy(
                                kh[:, kg4 * 512:(kg4 + 1) * 512], acc[:])
                        r0 = (h % 2) * ROPE
                        if h % 2 == 0:
                            oacc1 = ps_out()
                            oacc2 = ps_out()
                        else:
                            oacc1 = ps_pt(0)
                            oacc2 = ps_pt(1)
                        for kt in range(16):
                            nb = 4 - kt // 4
                            N = nb * 128
                            sc = ps_score()
                            nc.tensor.matmul(
                                sc[:, 0:N],
                                kh[:, kt * 128:(kt + 1) * 128],
                                qp[:, h, 0:N],
                                start=True, stop=False)
                            nc.tensor.matmul(
                                sc[:, 0:N],
                                krT[r0:r0 + ROPE, kt * 128:(kt + 1) * 128],
                                qr[r0:r0 + ROPE, h // 2, 0:N],
                                start=False, stop=True)
                            pt_ = ph.tile([128, 512], p_dt, tag="P", bufs=3)
                            nc.scalar.activation(out=pt_[:, 0:N],
                                                 in_=sc[:, 0:N], func=AF.Exp,
                                                 scale=SCALING)
                            nc.vector.tensor_mul(pt_[:, N - 128:N],
                                                 pt_[:, N - 128:N],
                                                 msk[:, kt, :])
                            nc.tensor.matmul(
                                oacc1[0:65, 0:N],
                                v4[:, kt, hh, 0, :],
                                pt_[:, 0:N],
                                start=(kt == 0), stop=(kt == 15),
                                skip_group_check=True)
                            nc.tensor.matmul(
                                oacc2[0:65, 0:N],
                                v4[:, kt, hh, 1, :],
                                pt_[:, 0:N],
                                start=(kt == 0), stop=(kt == 15),
                                skip_group_check=True)
                        srow = sm.tile([1, 512], F32, tag="srow", bufs=2)
                        nc.scalar.copy(srow[:], oacc1[64:65, :])
                        nc.vector.reciprocal(out=srow[:], in_=srow[:])
                        ib = ph.tile([128, 512], F32, tag="ib", bufs=2)
                        nc.gpsimd.partition_broadcast(ib[:], srow[0:1, :])
                        nc.vector.tensor_mul(at[0:64, h, :], oacc1[0:64, :],
                                             ib[0:64, :])
                        nc.vector.tensor_mul(at[64:128, h, :], oacc2[0:64, :],
                                             ib[0:64, :])

                # ============ Phase D: o-projection =====================
                for dq in range(4):
                    if dq == 0:
                        ow = ow0
                    else:
                        ow = ph.tile([128, HP, 512], mm_dt, tag="ow", bufs=2)
                        nc.sync.dma_start(
                            ow[:],
                            oWt[:, dq * 512:(dq + 1) * 512].rearrange(
                                "(hk p) c -> p hk c", p=128))
                    accs = [ps_pt(0), ps_pt(1), ps_pt(2), ps_out()]
                    for dt in range(4):
                        for hk in range(HP):
                            nc.tensor.matmul(
                                accs[dt][:],
                                ow[:, hk, dt * 128:(dt + 1) * 128],
                                at[:, hk, :],
                                start=(hk == 0), stop=(hk == HP - 1))
                    for dt in range(4):
                        ot = ph.tile([128, 512], F32, tag="ot", bufs=3)
                        if dt % 2 == 0:
                            nc.scalar.copy(ot[:], accs[dt][:])
                        else:
                            nc.vector.tensor_copy(ot[:], accs[dt][:])
                        d0 = dq * 512 + dt * 128
                        nc.sync.dma_start(outT[d0:d0 + 128, :], ot[:])

    nc.compile()
    return nc


def prep_inputs(inputs: dict, nheads: int = H) -> list[dict]:
    """Shard + pre-transpose the full inputs into 8 per-core input maps."""
    import ml_dtypes
    bf16 = ml_dtypes.bfloat16
    f32 = np.float32
    hs = np.asarray(inputs["hidden_states"], f32)
    cos = np.asarray(inputs["cos"], f32)
    sin = np.asarray(inputs["sin"], f32)
    qaW = np.asarray(inputs["q_a_W"], f32)
    qanw = np.asarray(inputs["q_a_norm_w"], f32)
    qbW = np.asarray(inputs["q_b_W"], f32)
    kvaW = np.asarray(inputs["kv_a_W"], f32)
    kvanw = np.asarray(inputs["kv_a_norm_w"], f32)
    kvbW = np.asarray(inputs["kv_b_W"], f32)
    oW = np.asarray(inputs["o_W"], f32)

    HP = nheads
    qaWt = np.ascontiguousarray(qaW.T)                      # [D, LQ]
    # fold q_a_norm_w into q_b rows (columns of q_b_W)
    qbWs = qbW[: HP * QKD] * qanw[None, :]                  # [HP*QKD, LQ]
    qb3 = qbWs.reshape(HP, QKD, LQ)
    qbWp = np.ascontiguousarray(
        qb3[:, :NOPE, :].reshape(HP * NOPE, LQ).T)          # [LQ, HP*128]
    qbWr = np.ascontiguousarray(
        qb3[:, NOPE:, :].reshape(HP * ROPE, LQ).T)          # [LQ, HP*64]
    kvaWt = np.ascontiguousarray(kvaW.T)                    # [D, 576]
    kvb3 = (kvbW[: HP * (NOPE + VDIM)] * kvanw[None, :]).reshape(
        HP, NOPE + VDIM, LKV)
    kvbWk = np.ascontiguousarray(
        kvb3[:, :NOPE, :].reshape(HP * NOPE, LKV).T)        # [LKV, HP*128]
    kvbWv = np.ascontiguousarray(
        kvb3[:, NOPE:, :].reshape(HP * VDIM, LKV).T)        # [LKV, HP*128]
    oWt = np.ascontiguousarray(oW[:, : HP * VDIM].T)        # [HP*128, D]

    qaWt = qaWt.astype(bf16)
    qbWp = qbWp.astype(bf16)
    qbWr = qbWr.astype(bf16)
    kvaWt = kvaWt.astype(bf16)
    kvbWk = kvbWk.astype(bf16)
    kvbWv = kvbWv.astype(bf16)
    oWt = oWt.astype(bf16)
    hTb = [np.ascontiguousarray(hs[b].T).astype(bf16) for b in range(B)]

    in_maps = []
    for c in range(NCORES):
        b, a = divmod(c, 4)
        blocks = _blocks_for(a)
        qidx = np.concatenate(
            [np.arange(j * 128, (j + 1) * 128) for j in blocks])
        hT = hTb[b]                                         # [D, S] bf16
        cosT = np.ascontiguousarray(cos[b].T)               # [64, S]
        sinT = np.ascontiguousarray(sin[b].T)
        sinneg = sinT.copy()
        sinneg[:32] = -sinneg[:32]
        cosq = cosT[:, qidx]
        sinq = sinneg[:, qidx]
        hTq = np.ascontiguousarray(hT[:, qidx])             # [D, NT] bf16
        mk = np.zeros((16, 128, 128), f32)
        for kt in range(16):
            t = 3 - kt // 4
            j = blocks[t]
            qpos = np.arange(j * 128, (j + 1) * 128)
            kpos = kt * 128 + np.arange(128)
            mk[kt] = (kpos[:, None] <= qpos[None, :]).astype(f32)
        in_maps.append({
            "hidT": hT,
            "hidTq": hTq,
            "qaWt": qaWt, "qbWp": qbWp, "qbWr": qbWr,
            "kvaWt": kvaWt, "kvbWk": kvbWk, "kvbWv": kvbWv, "oWt": oWt,
            "cosq2": np.ascontiguousarray(np.concatenate([cosq, cosq], 0)),
            "sinq2": np.ascontiguousarray(np.concatenate([sinq, sinq], 0)),
            "cosk": cosT, "sink": sinneg,
            "masks": mk.astype(bf16),
        })
    return in_maps


def assemble(results: list[dict]) -> np.ndarray:
    out = np.empty((B, S, D), np.float32)
    for c in range(NCORES):
        b, a = divmod(c, 4)
        blocks = _blocks_for(a)
        oT = results[c]["outT"]                             # [D, 512]
        for t, j in enumerate(blocks):
            out[b, j * 128:(j + 1) * 128, :] = oT[:, t * 128:(t + 1) * 128].T
    return out


_CACHE = {}


def _get_nc(nheads=H):
    key = nheads
    if key not in _CACHE:
        _CACHE[key] = build(nheads)
    return _CACHE[key]


def kernel(**inputs) -> np.ndarray:
    nc = _get_nc()
    in_maps = prep_inputs(inputs)
    res = run_bass_kernel_spmd(nc, in_maps, list(range(NCORES)))
    return assemble(res.results)



# revision 34
# speedup vs baseline: 1.0455x; 1.0362x over previous
"""DeepseekV3 MLA attention on 8 Trainium2 NeuronCores (Bass/Tile).

Sharding: token-parallel. Core c handles batch b = c//4 and 512 query tokens
(4 chunks of 128, zig-zag {a, 7-a, 8+a, 15-a} with a = c%4, ordered by
descending causal-prefix length so every core runs the identical program).
Each core computes the full KV path for its batch (replicated within the
4-core batch group), attention for all 16 heads over its queries, and the
full o-projection for its tokens.  No collectives; host unsharding is pure
concatenation/transposition.

Layouts are transposed (feature-on-partition) end to end; weights arrive
host-pre-transposed; RoPE runs in transposed layout using partition-shifted
single-input copies; softmax is unstabilized exp (scores are ~N(0,1) after
the 1/sqrt(192) scaling, bounded far below f32 overflow); causal masking is
data-driven (per-slot mask tiles multiply probabilities after exp).
"""

import numpy as np

import concourse.bass as bass
import concourse.mybir as mybir
import concourse.tile as tile
from concourse import bacc
from concourse.masks import make_identity
from concourse.bass_utils import run_bass_kernel_spmd

# Problem constants (hardcoded per contract).
B, S, D = 2, 2048, 2048
H = 16
LQ = 1536           # q low-rank dim
LKV = 512           # kv latent dim
ROPE = 64
NOPE = 128
VDIM = 128
QKD = NOPE + ROPE   # 192
EPS = 1e-6
SCALING = QKD ** -0.5
NT = 512            # query tokens per core
NCORES = 8

F32 = mybir.dt.float32
F32R = mybir.dt.float32r
BF16 = mybir.dt.bfloat16
AF = mybir.ActivationFunctionType
ALU = mybir.AluOpType
AX = mybir.AxisListType


def _blocks_for(a: int) -> list[int]:
    """Query chunk (of 128 tokens) handled by col-block t, t=0..3.

    Block t has causal key prefix covering key groups kg=0..3-t (512 keys
    each); chunk ids are chosen so prefixes are compatible: J_t in
    [12-4t, 15-4t].
    """
    return [15 - a, 8 + a, 7 - a, a]


def build(nheads: int = H, mm_dt=BF16, p_dt=BF16):
    """Build the SPMD Bass program."""
    HP = nheads
    NPAIR = (HP + 1) // 2
    HG = (HP + 3) // 4
    GSZ = min(4, HP)
    NLQ = LQ // 128      # 12
    NLKV = LKV // 128    # 4

    nc = bacc.Bacc("TRN2", target_bir_lowering=False, debug=False,
                   num_devices=NCORES)

    def din(name, shape, dt=mm_dt):
        return nc.dram_tensor(name, list(shape), dt, kind="ExternalInput")

    hidT = din("hidT", [D, S])
    hidTq = din("hidTq", [D, NT])
    qaWt = din("qaWt", [D, LQ])
    qbWp = din("qbWp", [LQ, HP * NOPE])
    qbWr = din("qbWr", [LQ, HP * ROPE])
    kvaWt = din("kvaWt", [D, LKV + ROPE])
    kvbWk = din("kvbWk", [LKV, HP * NOPE])
    kvbWv = din("kvbWv", [LKV, HP * VDIM])
    oWt = din("oWt", [HP * VDIM, D])
    cosq2 = din("cosq2", [2 * ROPE, NT], F32)
    sinq2 = din("sinq2", [2 * ROPE, NT], F32)   # first half negated
    cosk = din("cosk", [ROPE, S], F32)
    sink = din("sink", [ROPE, S], F32)          # first half negated
    masks = din("masks", [16, 128, 128])
    outT = nc.dram_tensor("outT", [D, NT], F32, kind="ExternalOutput")

    with tile.TileContext(nc) as tc:
        with (
            tc.tile_pool(name="psum", bufs=1, space="PSUM") as pp,
            tc.tile_pool(name="peC", bufs=1) as peC,
            tc.tile_pool(name="small", bufs=1) as sm,
        ):
            def ps_score():
                return pp.tile([128, 512], F32, tag="score", bufs=2,
                               name="ps_score")

            def ps_pt(i, dt=F32):
                return pp.tile([128, 512], dt, tag=f"pt{i}", bufs=1,
                               name=f"ps_pt{i}")

            def ps_out():
                return pp.tile([128, 512], F32, tag="out", bufs=2,
                               name="ps_out")

            def ps_misc():
                return pp.tile([128, 512], F32, tag="misc", bufs=1,
                               name="ps_misc")

            def ps_tp(dt):
                return pp.tile([128, 512], dt, tag="score", bufs=2,
                               name="ps_tp")

            # identities, eps
            ident_f = sm.tile([128, 128], F32)
            make_identity(nc, ident_f[:])
            ident_r = sm.tile([128, 128], mm_dt)
            nc.vector.tensor_copy(ident_r[:], ident_f[:])
            epsb = sm.tile([128, 1], F32)
            nc.vector.memset(epsb[:], EPS)

            # Long-lived (through phase C) tensors.
            qp = peC.tile([128, HP, NT], mm_dt)      # q_pass^T per head
            vw0 = peC.tile([128, NLKV, GSZ * VDIM], mm_dt)   # C hg=0 V wts
            kwg0 = peC.tile([128, NLKV, GSZ * NOPE], mm_dt)  # C hg=0 K wts
            qr = peC.tile([128, NPAIR, NT], mm_dt)   # roped q_rot^T head pairs
            krT = peC.tile([128, S], mm_dt)          # roped k_rot^T (dup halves)
            cT = peC.tile([128, NLKV, S], mm_dt)     # c-tilde^T
            at = peC.tile([128, HP, NT], mm_dt)      # attn out, head-major

            # ============ Phase A-q: q_a + rmsnorm + transpose ==========
            with tc.tile_pool(name="phaq", bufs=1) as ph:
                qT = ph.tile([128, NLQ, NT], mm_dt)
                # next-phase tiles, DMA-prefetched during A-q compute
                kvw = ph.tile([128, 16, LKV + ROPE], mm_dt)
                ck = ph.tile([ROPE, S], F32, tag="ck")
                sk = ph.tile([ROPE, S], F32, tag="sk")
                hid0 = ph.tile([128, 8, 512], mm_dt, tag="hid0")
                w0 = ph.tile([128, NLQ, 2 * NOPE], mm_dt, tag="qbw0")
                cq = ph.tile([2 * ROPE, NT], F32, tag="cq")
                sq2 = ph.tile([2 * ROPE, NT], F32, tag="sq2")
                with tc.tile_pool(name="phaq2", bufs=1) as ph2:
                    hq = ph2.tile([128, 16, NT], mm_dt)
                    nc.sync.dma_start(
                        hq[:], hidTq.rearrange("(dk p) t -> p dk t", p=128))
                    qnat = ph2.tile([128, 4, LQ], mm_dt)
                    ssq = ph2.tile([128, 4], F32)  # per-tq sum of squares
                    for g in range(3):
                        accs = [ps_pt(0), ps_pt(1), ps_pt(2), ps_out()]
                        for dk2 in range(8):
                            qaw = ph2.tile([128, 2, 512], mm_dt, tag="qaw",
                                           bufs=3)
                            nc.sync.dma_start(
                                qaw[:],
                                qaWt[dk2 * 256:(dk2 + 1) * 256,
                                     g * 512:(g + 1) * 512].rearrange(
                                    "(two p) c -> p two c", p=128))
                            for ti in range(2):
                                dk = dk2 * 2 + ti
                                for tq in range(4):
                                    nc.tensor.matmul(
                                        accs[tq][:],
                                        hq[:, dk, tq * 128:(tq + 1) * 128],
                                        qaw[:, ti, :],
                                        start=(dk == 0), stop=(dk == 15))
                        for tq in range(4):
                            sq = ph2.tile([128, 512], F32, tag="sq", bufs=2)
                            nc.scalar.activation(out=sq[:], in_=accs[tq][:],
                                                 func=AF.Square)
                            ps = sm.tile([128, 1], F32, tag="ss", bufs=4)
                            nc.vector.reduce_sum(out=ps[:], in_=sq[:],
                                                 axis=AX.X)
                            if g == 0:
                                nc.vector.tensor_copy(ssq[:, tq:tq + 1],
                                                      ps[:])
                            else:
                                nc.vector.tensor_add(ssq[:, tq:tq + 1],
                                                     ssq[:, tq:tq + 1], ps[:])
                            nc.vector.tensor_copy(
                                qnat[:, tq, g * 512:(g + 1) * 512], accs[tq][:])
                        if g == 0:
                            # prefetch A-kv inputs while g=1/2 compute
                            nc.sync.dma_start(
                                kvw[:],
                                kvaWt.rearrange("(dk p) c -> p dk c", p=128))
                            nc.sync.dma_start(
                                hid0[:],
                                hidT[0:1024, 0:512].rearrange(
                                    "(dk p) t -> p dk t", p=128))
                    # prefetch rope tables + B-q first weight tile
                    nc.sync.dma_start(ck[:], cosk[:])
                    nc.sync.dma_start(sk[:], sink[:])
                    nc.sync.dma_start(
                        w0[:],
                        qbWp[:, 0:2 * NOPE].rearrange("(lk p) c -> p lk c",
                                                      p=128))
                    nc.sync.dma_start(cq[:], cosq2[:])
                    nc.sync.dma_start(sq2[:], sinq2[:])
                    # rsqrt + scale + transpose
                    nc.scalar.activation(out=ssq[:], in_=ssq[:], func=AF.Sqrt,
                                         bias=epsb[:], scale=1.0 / LQ)
                    nc.vector.reciprocal(out=ssq[:], in_=ssq[:])
                    for tq in range(4):
                        nc.vector.tensor_scalar(
                            out=qnat[:, tq, :], in0=qnat[:, tq, :],
                            scalar1=ssq[:, tq:tq + 1], scalar2=None,
                            op0=ALU.mult)
                    for lk in range(NLQ):
                        tp = ps_tp(mm_dt)
                        for tq in range(4):
                            nc.tensor.transpose(
                                tp[:, tq * 128:(tq + 1) * 128],
                                qnat[:, tq, lk * 128:(lk + 1) * 128],
                                ident_r[:])
                        nc.scalar.copy(qT[:, lk, :], tp[:])

                # ==== Phase A-kv: kv_a + rmsnorm + k-rope + transpose ===
                with tc.tile_pool(name="phakv", bufs=1) as ph4:
                    for q4 in range(4):
                        accs = [ps_pt(0), ps_pt(1), ps_pt(2), ps_out()]
                        rot = ps_misc()
                        for half in range(2):
                            if q4 == 0 and half == 0:
                                hid = hid0
                            else:
                                hid = ph4.tile([128, 8, 512], mm_dt,
                                               tag="hid", bufs=2)
                                nc.sync.dma_start(
                                    hid[:],
                                    hidT[half * 1024:(half + 1) * 1024,
                                         q4 * 512:(q4 + 1) * 512].rearrange(
                                        "(dk p) t -> p dk t", p=128))
                            for dk8 in range(8):
                                dk = half * 8 + dk8
                                for tk in range(4):
                                    nc.tensor.matmul(
                                        accs[tk][:],
                                        hid[:, dk8, tk * 128:(tk + 1) * 128],
                                        kvw[:, dk, 0:LKV],
                                        start=(dk == 0), stop=(dk == 15))
                                nc.tensor.matmul(rot[0:ROPE, :],
                                                 kvw[:, dk, LKV:LKV + ROPE],
                                                 hid[:, dk8, :],
                                                 start=(dk == 0),
                                                 stop=(dk == 15))
                        for tk in range(4):
                            tt = q4 * 4 + tk
                            sq = ph4.tile([128, LKV], F32, tag="sqkv", bufs=2)
                            nc.scalar.activation(out=sq[:], in_=accs[tk][:],
                                                 func=AF.Square)
                            ss = sm.tile([128, 1], F32, tag="ss", bufs=4)
                            nc.vector.reduce_sum(out=ss[:], in_=sq[:],
                                                 axis=AX.X)
                            nc.scalar.activation(out=ss[:], in_=ss[:],
                                                 func=AF.Sqrt, bias=epsb[:],
                                                 scale=1.0 / LKV)
                            nc.vector.reciprocal(out=ss[:], in_=ss[:])
                            cnt = ph4.tile([128, LKV], mm_dt, tag="cnt",
                                           bufs=3)
                            nc.vector.tensor_scalar(
                                out=cnt[:], in0=accs[tk][:],
                                scalar1=ss[:], scalar2=None, op0=ALU.mult)
                            tp = ps_tp(mm_dt)
                            for lk in range(NLKV):
                                nc.tensor.transpose(
                                    tp[:, lk * 128:(lk + 1) * 128],
                                    cnt[:, lk * 128:(lk + 1) * 128],
                                    ident_r[:])
                            nc.scalar.copy(
                                cT[:, :, tt * 128:(tt + 1) * 128],
                                tp[:].rearrange("p (lk c) -> p lk c", c=128))
                        kR = ph4.tile([ROPE, 512], F32, tag="kR", bufs=2)
                        kS = ph4.tile([ROPE, 512], F32, tag="kS", bufs=2)
                        nc.scalar.copy(kR[:], rot[0:ROPE, :])
                        nc.scalar.copy(kS[0:32, :], rot[32:64, :])
                        nc.scalar.copy(kS[32:64, :], rot[0:32, :])
                        cs = ck[:, q4 * 512:(q4 + 1) * 512]
                        sn = sk[:, q4 * 512:(q4 + 1) * 512]
                        nc.vector.tensor_mul(kR[:], kR[:], cs)
                        nc.vector.tensor_mul(kS[:], kS[:], sn)
                        nc.vector.tensor_add(
                            krT[0:ROPE, q4 * 512:(q4 + 1) * 512],
                            kR[:], kS[:])
                        nc.vector.tensor_add(
                            krT[ROPE:2 * ROPE, q4 * 512:(q4 + 1) * 512],
                            kR[:], kS[:])

                # ============ Phase B-q: q_b + q-rope (uses qT) =========
                with tc.tile_pool(name="phbq", bufs=1) as ph3:
                    for h2 in range((HP + 1) // 2):
                        nh = min(2, HP - h2 * 2)
                        if h2 == 0:
                            w = w0
                        else:
                            w = ph3.tile([128, NLQ, 2 * NOPE], mm_dt,
                                         tag="qbw", bufs=3)
                            nc.sync.dma_start(
                                w[:, :, 0:nh * NOPE],
                                qbWp[:, h2 * 2 * NOPE:
                                     (h2 * 2 + nh) * NOPE].rearrange(
                                    "(lk p) c -> p lk c", p=128))
                        for hh in range(nh):
                            h = h2 * 2 + hh
                            acc = ps_score() if h % 2 == 0 else ps_out()
                            for lk in range(NLQ):
                                nc.tensor.matmul(
                                    acc[:],
                                    w[:, lk, hh * NOPE:(hh + 1) * NOPE],
                                    qT[:, lk, :],
                                    start=(lk == 0), stop=(lk == NLQ - 1))
                            if h % 2 == 0:
                                nc.scalar.copy(qp[:, h, :], acc[:])
                            else:
                                nc.vector.tensor_copy(qp[:, h, :], acc[:])
                    for p in range(NPAIR):
                        npr = min(2 * ROPE, HP * ROPE - p * 2 * ROPE)
                        w = ph3.tile([128, NLQ, 2 * ROPE], mm_dt, tag="qrw",
                                     bufs=3)
                        nc.sync.dma_start(
                            w[:, :, 0:npr],
                            qbWr[:, p * 2 * ROPE:p * 2 * ROPE + npr].rearrange(
                                "(lk p) c -> p lk c", p=128))
                        acc = ps_pt(p % 3)
                        for lk in range(NLQ):
                            nc.tensor.matmul(acc[0:npr, :], w[:, lk, 0:npr],
                                             qT[:, lk, :],
                                             start=(lk == 0),
                                             stop=(lk == NLQ - 1))
                        qR = ph3.tile([128, NT], F32, tag="qR", bufs=2)
                        qS = ph3.tile([128, NT], F32, tag="qS", bufs=2)
                        for hh in range(npr // ROPE):
                            r0 = hh * ROPE
                            nc.scalar.copy(qS[r0:r0 + 32, :],
                                           acc[r0 + 32:r0 + 64, :])
                            nc.scalar.copy(qS[r0 + 32:r0 + 64, :],
                                           acc[r0:r0 + 32, :])
                        nc.vector.tensor_mul(qR[0:npr, :], acc[0:npr, :],
                                             cq[0:npr, :])
                        nc.vector.tensor_mul(qS[0:npr, :], qS[0:npr, :],
                                             sq2[0:npr, :])
                        nc.vector.tensor_add(qr[0:npr, p, :], qR[0:npr, :],
                                             qS[0:npr, :])
                    # prefetch phase-C first head-group weights
                    nc.sync.dma_start(
                        vw0[:],
                        kvbWv[:, 0:GSZ * VDIM].rearrange(
                            "(lk p) c -> p lk c", p=128))
                    nc.sync.dma_start(
                        kwg0[:],
                        kvbWk[:, 0:GSZ * NOPE].rearrange(
                            "(lk p) c -> p lk c", p=128))

            # ============ Phase C: attention ============================
            # Transposed scores: S^T[k, q] = K @ Q^T per 128-key tile; AV
            # consumes P~^T directly (lhsT = [V-half | ones]), yielding the
            # softmax denominator as PSUM row 64 for free.
            with tc.tile_pool(name="phc", bufs=1) as ph:
                msk = ph.tile([128, 16, 128], mm_dt)
                nc.sync.dma_start(msk[:], masks.rearrange("t p c -> p t c"))
                ones1 = sm.tile([128, 1], F32)
                nc.vector.memset(ones1[:], 1.0)
                ow0 = ph.tile([128, HP, 512], mm_dt, tag="ow0")
                for hg in range(HG):
                    if hg == 1:
                        nc.sync.dma_start(
                            ow0[:],
                            oWt[:, 0:512].rearrange("(hk p) c -> p hk c",
                                                    p=128))
                    h0 = hg * GSZ
                    gs = min(GSZ, HP - h0)
                    if hg == 0:
                        vw = vw0
                        kwg = kwg0
                    else:
                        vw = ph.tile([128, NLKV, GSZ * VDIM], mm_dt,
                                     tag="vw", bufs=2)
                        nc.sync.dma_start(
                            vw[:, :, 0:gs * VDIM],
                            kvbWv[:, h0 * VDIM:(h0 + gs) * VDIM].rearrange(
                                "(lk p) c -> p lk c", p=128))
                        kwg = ph.tile([128, NLKV, GSZ * NOPE], mm_dt,
                                      tag="kwg", bufs=2)
                        nc.sync.dma_start(
                            kwg[:, :, 0:gs * NOPE],
                            kvbWk[:, h0 * NOPE:(h0 + gs) * NOPE].rearrange(
                                "(lk p) c -> p lk c", p=128))
                    v4 = ph.tile([128, 16, GSZ, 2, 65], mm_dt, tag="v4",
                                 bufs=1)
                    nc.vector.tensor_copy(
                        v4[:, :, 0:gs, :, 64:65],
                        ones1[:].to_broadcast([128, 16, gs, 2, 1]))
                    for tt in range(16):
                        acc = ps_misc() if tt % 2 == 0 else ps_pt(2)
                        for lk in range(NLKV):
                            nc.tensor.matmul(
                                acc[:, 0:gs * VDIM],
                                cT[:, lk, tt * 128:(tt + 1) * 128],
                                vw[:, lk, 0:gs * VDIM],
                                start=(lk == 0), stop=(lk == NLKV - 1))
                        nc.scalar.copy(
                            v4[:, tt, 0:gs, :, 0:64],
                            acc[:, 0:gs * VDIM].rearrange(
                                "p (g two s) -> p g two s", two=2, s=64))
                    for hh in range(gs):
                        h = h0 + hh
                        kh = ph.tile([128, S], mm_dt, tag="kh", bufs=2)
                        for kg4 in range(4):
                            acc = ps_misc() if kg4 % 2 == 0 else ps_pt(2)
                            for lk in range(NLKV):
                                nc.tensor.matmul(
                                    acc[:],
                                    kwg[:, lk, hh * NOPE:(hh + 1) * NOPE],
                                    cT[:, lk, kg4 * 512:(kg4 + 1) * 512],
                                    start=(lk == 0), stop=(lk == NLKV - 1))
                            nc.vector.tensor_copy(
                                kh[:, kg4 * 512:(kg4 + 1) * 512], acc[:])
                        r0 = (h % 2) * ROPE
                        if h % 2 == 0:
                            oacc1 = ps_out()
                            oacc2 = ps_out()
                        else:
                            oacc1 = ps_pt(0)
                            oacc2 = ps_pt(1)
                        for kt in range(16):
                            nb = 4 - kt // 4
                            N = nb * 128
                            sc = ps_score()
                            nc.tensor.matmul(
                                sc[:, 0:N],
                                kh[:, kt * 128:(kt + 1) * 128],
                                qp[:, h, 0:N],
                                start=True, stop=False)
                            nc.tensor.matmul(
                                sc[:, 0:N],
                                krT[r0:r0 + ROPE, kt * 128:(kt + 1) * 128],
                                qr[r0:r0 + ROPE, h // 2, 0:N],
                                start=False, stop=True)
                            pt_ = ph.tile([128, 512], p_dt, tag="P", bufs=3)
                            nc.scalar.activation(out=pt_[:, 0:N],
                                                 in_=sc[:, 0:N], func=AF.Exp,
                                                 scale=SCALING)
                            nc.vector.tensor_mul(pt_[:, N - 128:N],
                                                 pt_[:, N - 128:N],
                                                 msk[:, kt, :])
                            nc.tensor.matmul(
                                oacc1[0:65, 0:N],
                                v4[:, kt, hh, 0, :],
                                pt_[:, 0:N],
                                start=(kt == 0), stop=(kt == 15),
                                skip_group_check=True)
                            nc.tensor.matmul(
                                oacc2[0:65, 0:N],
                                v4[:, kt, hh, 1, :],
                                pt_[:, 0:N],
                                start=(kt == 0), stop=(kt == 15),
                                skip_group_check=True)
                        srow = sm.tile([1, 512], F32, tag="srow", bufs=2)
                        nc.scalar.copy(srow[:], oacc1[64:65, :])
                        nc.vector.reciprocal(out=srow[:], in_=srow[:])
                        ib = ph.tile([128, 512], F32, tag="ib", bufs=2)
                        nc.gpsimd.partition_broadcast(ib[:], srow[0:1, :])
                        nc.vector.tensor_mul(at[0:64, h, :], oacc1[0:64, :],
                                             ib[0:64, :])
                        nc.vector.tensor_mul(at[64:128, h, :], oacc2[0:64, :],
                                             ib[0:64, :])

                # ============ Phase D: o-projection =====================
                for dq in range(4):
                    if dq == 0:
                        ow = ow0
                    else:
                        ow = ph.tile([128, HP, 512], mm_dt, tag="ow", bufs=2)
                        nc.sync.dma_start(
                            ow[:],
                            oWt[:, dq * 512:(dq + 1) * 512].rearrange(
                                "(hk p) c -> p hk c", p=128))
                    accs = [ps_pt(0), ps_pt(1), ps_pt(2), ps_out()]
                    for dt in range(4):
                        for hk in range(HP):
                            nc.tensor.matmul(
                                accs[dt][:],
                                ow[:, hk, dt * 128:(dt + 1) * 128],
                                at[:, hk, :],
                                start=(hk == 0), stop=(hk == HP - 1))
                    for dt in range(4):
                        ot = ph.tile([128, 512], F32, tag="ot", bufs=3)
                        if dt % 2 == 0:
                            nc.scalar.copy(ot[:], accs[dt][:])
                        else:
                            nc.vector.tensor_copy(ot[:], accs[dt][:])
                        d0 = dq * 512 + dt * 128
                        nc.sync.dma_start(outT[d0:d0 + 128, :], ot[:])

    nc.compile()
    return nc


def prep_inputs(inputs: dict, nheads: int = H) -> list[dict]:
    """Shard + pre-transpose the full inputs into 8 per-core input maps."""
    import ml_dtypes
    bf16 = ml_dtypes.bfloat16
    f32 = np.float32
    hs = np.asarray(inputs["hidden_states"], f32)
    cos = np.asarray(inputs["cos"], f32)
    sin = np.asarray(inputs["sin"], f32)
    qaW = np.asarray(inputs["q_a_W"], f32)
    qanw = np.asarray(inputs["q_a_norm_w"], f32)
    qbW = np.asarray(inputs["q_b_W"], f32)
    kvaW = np.asarray(inputs["kv_a_W"], f32)
    kvanw = np.asarray(inputs["kv_a_norm_w"], f32)
    kvbW = np.asarray(inputs["kv_b_W"], f32)
    oW = np.asarray(inputs["o_W"], f32)

    HP = nheads
    qaWt = np.ascontiguousarray(qaW.T)                      # [D, LQ]
    # fold q_a_norm_w into q_b rows (columns of q_b_W)
    qbWs = qbW[: HP * QKD] * qanw[None, :]                  # [HP*QKD, LQ]
    qb3 = qbWs.reshape(HP, QKD, LQ)
    qbWp = np.ascontiguousarray(
        qb3[:, :NOPE, :].reshape(HP * NOPE, LQ).T)          # [LQ, HP*128]
    qbWr = np.ascontiguousarray(
        qb3[:, NOPE:, :].reshape(HP * ROPE, LQ).T)          # [LQ, HP*64]
    kvaWt = np.ascontiguousarray(kvaW.T)                    # [D, 576]
    kvb3 = (kvbW[: HP * (NOPE + VDIM)] * kvanw[None, :]).reshape(
        HP, NOPE + VDIM, LKV)
    kvbWk = np.ascontiguousarray(
        kvb3[:, :NOPE, :].reshape(HP * NOPE, LKV).T)        # [LKV, HP*128]
    kvbWv = np.ascontiguousarray(
        kvb3[:, NOPE:, :].reshape(HP * VDIM, LKV).T)        # [LKV, HP*128]
    oWt = np.ascontiguousarray(oW[:, : HP * VDIM].T)        # [HP*128, D]

    qaWt = qaWt.astype(bf16)
    qbWp = qbWp.astype(bf16)
    qbWr = qbWr.astype(bf16)
    kvaWt = kvaWt.astype(bf16)
    kvbWk = kvbWk.astype(bf16)
    kvbWv = kvbWv.astype(bf16)
    oWt = oWt.astype(bf16)
    hTb = [np.ascontiguousarray(hs[b].T).astype(bf16) for b in range(B)]

    in_maps = []
    for c in range(NCORES):
        b, a = divmod(c, 4)
        blocks = _blocks_for(a)
        qidx = np.concatenate(
            [np.arange(j * 128, (j + 1) * 128) for j in blocks])
        hT = hTb[b]                                         # [D, S] bf16
        cosT = np.ascontiguousarray(cos[b].T)               # [64, S]
        sinT = np.ascontiguousarray(sin[b].T)
        sinneg = sinT.copy()
        sinneg[:32] = -sinneg[:32]
        cosq = cosT[:, qidx]
        sinq = sinneg[:, qidx]
        hTq = np.ascontiguousarray(hT[:, qidx])             # [D, NT] bf16
        mk = np.zeros((16, 128, 128), f32)
        for kt in range(16):
            t = 3 - kt // 4
            j = blocks[t]
            qpos = np.arange(j * 128, (j + 1) * 128)
            kpos = kt * 128 + np.arange(128)
            mk[kt] = (kpos[:, None] <= qpos[None, :]).astype(f32)
        in_maps.append({
            "hidT": hT,
            "hidTq": hTq,
            "qaWt": qaWt, "qbWp": qbWp, "qbWr": qbWr,
            "kvaWt": kvaWt, "kvbWk": kvbWk, "kvbWv": kvbWv, "oWt": oWt,
            "cosq2": np.ascontiguousarray(np.concatenate([cosq, cosq], 0)),
            "sinq2": np.ascontiguousarray(np.concatenate([sinq, sinq], 0)),
            "cosk": cosT, "sink": sinneg,
            "masks": mk.astype(bf16),
        })
    return in_maps


def assemble(results: list[dict]) -> np.ndarray:
    out = np.empty((B, S, D), np.float32)
    for c in range(NCORES):
        b, a = divmod(c, 4)
        blocks = _blocks_for(a)
        oT = results[c]["outT"]                             # [D, 512]
        for t, j in enumerate(blocks):
            out[b, j * 128:(j + 1) * 128, :] = oT[:, t * 128:(t + 1) * 128].T
    return out


_CACHE = {}


def _get_nc(nheads=H):
    key = nheads
    if key not in _CACHE:
        _CACHE[key] = build(nheads)
    return _CACHE[key]


def kernel(**inputs) -> np.ndarray:
    nc = _get_nc()
    in_maps = prep_inputs(inputs)
    res = run_bass_kernel_spmd(nc, in_maps, list(range(NCORES)))
    return assemble(res.results)

